# revision 31
# baseline (speedup 1.0000x reference)
"""GAT layer on 8 TRN2 cores: dst-sharded, edge-gather + one-hot segment matmul.

Design (v2):
  - Output nodes (dst) sharded contiguously across 8 cores (NPC nodes each).
  - Phase 1: each core computes the FULL transformed-feature table
    h = x @ W_ext (264 cols: 256 h + 4 a_src + 4 a_dst logits, single matmul)
    written to its DRAM as 768B rows. Lo half ([0,SPLIT)) writes are fenced
    separately from hi half so phase-2 lo gathers overlap the hi table build.
    Phase 1b computes a_dst logits for its own node range (SBUF resident).
  - Phase 2: per (128-dst window, lo/hi stream) the edges are gathered from
    the table via SWDGE dma_gather with per-(w,s) tile counts (T_ws = max
    over cores, not global max), scored exp(leakyrelu(a_src + a_dst)),
    scaled, and segment-summed into PSUM via per-tile one-hot matmuls
    (fp8 one-hots from host). Denominators ride in columns 256:260.
  - Self-loops are not gathered: they are applied from the SBUF-resident
    own-range table at normalize time. All phase-1/phase-2 tile pools are
    opened in one scope so SBUF zones stay disjoint (zone reuse would
    serialize phase 2 behind every phase-1 read).
"""
import sys
sys.path.insert(0, '/opt/trn_rl_repo')
from contextlib import ExitStack
import numpy as np
import ml_dtypes

import bass_rust as _br
import concourse.bacc as bacc
import concourse.mybir as mybir
import concourse.tile as tile
from concourse import bass_utils

BF16 = ml_dtypes.bfloat16
FP8 = ml_dtypes.float8_e4m3

C_IN = 128
C_OUT_TOT = 256   # HEADS * OUT_CH
HEADS = 4
HC = 64
NEG_SLOPE = 0.2
ROW = 384         # table row: 256 h + 4 a_src + 124 pad (bf16) = 768 B


def host_prep(x, edge_index, W, att_src, att_dst, bias, n_cores=8):
    """Shard + schedule. Returns (cfg, in_maps)."""
    N = x.shape[0]
    # self-loops are NOT gathered: served from SBUF-resident own_h at
    # normalize time.
    src = np.asarray(edge_index[0], np.int64).astype(np.int32)
    dst = np.asarray(edge_index[1], np.int64).astype(np.int32)

    NPC = N // n_cores
    NW = (NPC + 127) // 128
    SPLIT = (N + 1) // 2
    assert SPLIT < 32768 and (N - SPLIT) < 32768

    # per (core, window, stream) edge lists
    lists = [[[None, None] for _ in range(NW)] for _ in range(n_cores)]
    order = np.argsort(dst, kind='stable')
    src_s, dst_s = src[order], dst[order]
    for c in range(n_cores):
        lo_c = np.searchsorted(dst_s, c * NPC, 'left')
        hi_c = np.searchsorted(dst_s, (c + 1) * NPC, 'left')
        sc, dc = src_s[lo_c:hi_c], dst_s[lo_c:hi_c]
        dl = dc - c * NPC
        for w in range(NW):
            m = (dl >= w * 128) & (dl < (w + 1) * 128)
            sw, dw = sc[m], dl[m] - w * 128
            for s in range(2):
                ms = (sw < SPLIT) if s == 0 else (sw >= SPLIT)
                # ascending-src order within each call: SWDGE desc-gen is
                # address-sensitive (measured 8.1 vs 10.2 ns/idx), nearby
                # indices generate descriptors faster. Order is semantically
                # free — the one-hots encode each edge's slot.
                ss, dd = sw[ms], dw[ms]
                o2 = np.argsort(ss, kind='stable')
                lists[c][w][s] = (ss[o2], dd[o2])

    # per-(w,s) tile counts = max over cores
    T_ws = np.zeros((NW, 2), np.int32)
    for w in range(NW):
        for s in range(2):
            n_max = max(len(lists[c][w][s][0]) for c in range(n_cores))
            T_ws[w, s] = max(1, (n_max + 127) // 128)
    # slot offsets (in slots of 128) per (w, s), laid out w-major, s-minor
    off_ws = np.zeros((NW, 2), np.int64)
    acc = 0
    for w in range(NW):
        for s in range(2):
            off_ws[w, s] = acc
            acc += int(T_ws[w, s]) * 128
    TOT = int(acc)          # total gather slots per core
    assert TOT % 16 == 0

    cfg = dict(N=N, n_cores=n_cores, NPC=NPC, NW=NW, SPLIT=SPLIT,
               T_ws=T_ws, off_ws=off_ws, TOT=TOT)

    xT = np.ascontiguousarray(x.T).astype(BF16)            # [128, N]
    W_b = np.asarray(W, np.float32).astype(BF16)           # [128, 256]
    WT_b = np.ascontiguousarray(np.asarray(W).T).astype(BF16)  # [256, 128]
    att_flatT = np.zeros((C_OUT_TOT, 2 * HEADS), np.float32)
    for h in range(HEADS):
        att_flatT[h * HC:(h + 1) * HC, h] = np.asarray(att_src)[h]
        att_flatT[h * HC:(h + 1) * HC, HEADS + h] = np.asarray(att_dst)[h]
    att_flatT_b = att_flatT.astype(BF16)                   # [256, 8]
    bias_bc = np.broadcast_to(np.asarray(bias, np.float32), (128, C_OUT_TOT)).copy()

    in_maps = []
    for c in range(n_cores):
        idx16 = np.zeros((128, TOT // 16), np.int16)
        ohT = np.zeros((128, TOT), FP8)   # [e_lane, slot-space dst cols]
        ohF = np.zeros((128, TOT), FP8)   # [dst_lane, slot-space e cols]
        for w in range(NW):
            for s in range(2):
                ssw, sdw = lists[c][w][s]
                n = len(ssw)
                ts = int(T_ws[w, s]) * 128
                o = int(off_ws[w, s])
                idx = np.zeros(ts, np.int16)
                idx[:n] = (ssw - (SPLIT if s else 0)).astype(np.int16)
                wrapped = idx.reshape(ts // 16, 16).T
                idx16[:, o // 16:(o + ts) // 16] = np.tile(wrapped, (8, 1))
                e_pos = np.arange(n)
                lanes = e_pos % 128
                tiles = e_pos // 128
                ohT[lanes, o + tiles * 128 + sdw] = 1.0
                ohF[sdw, o + tiles * 128 + lanes] = 1.0
        in_maps.append({
            "xT": xT, "xT_own": np.ascontiguousarray(xT[:, c * NPC:(c + 1) * NPC]),
            "Wb": W_b, "WTb": WT_b, "attT": att_flatT_b, "bias_bc": bias_bc,
            "idx16": idx16, "ohT": ohT, "ohF": ohF,
        })
    return cfg, in_maps


def build_program(cfg):
    N, NPC, NW, SPLIT = (cfg[k] for k in ("N", "NPC", "NW", "SPLIT"))
    T_ws, off_ws, TOT = cfg["T_ws"], cfg["off_ws"], cfg["TOT"]
    n_cores = cfg["n_cores"]
    T_CAP = int(T_ws.max())
    dt = mybir.dt

    nc = bacc.Bacc("TRN2", target_bir_lowering=False, debug=False,
                   num_devices=n_cores)
    t_xT = nc.dram_tensor("xT", (128, N), dt.bfloat16, kind="ExternalInput")
    t_xT_own = nc.dram_tensor("xT_own", (128, NPC), dt.bfloat16, kind="ExternalInput")
    t_Wb = nc.dram_tensor("Wb", (C_IN, C_OUT_TOT), dt.bfloat16, kind="ExternalInput")
    t_WTb = nc.dram_tensor("WTb", (C_OUT_TOT, C_IN), dt.bfloat16, kind="ExternalInput")
    t_attT = nc.dram_tensor("attT", (C_OUT_TOT, 2 * HEADS), dt.bfloat16, kind="ExternalInput")
    t_bias = nc.dram_tensor("bias_bc", (128, C_OUT_TOT), dt.float32, kind="ExternalInput")
    t_idx = nc.dram_tensor("idx16", (128, TOT // 16), dt.int16, kind="ExternalInput")
    t_ohT = nc.dram_tensor("ohT", (128, TOT), dt.float8e4, kind="ExternalInput")
    t_ohF = nc.dram_tensor("ohF", (128, TOT), dt.float8e4, kind="ExternalInput")
    t_htab = nc.dram_tensor("htab", (N, ROW), dt.bfloat16, kind="Internal")
    t_out = nc.dram_tensor("out", (NPC, C_OUT_TOT), dt.float32, kind="ExternalOutput")

    with tile.TileContext(nc) as tc:
        with tc.tile_pool(name="const", bufs=1) as cpool, ExitStack() as stack:
            bias_sb = cpool.tile([128, C_OUT_TOT], dt.float32)
            nc.sync.dma_start(out=bias_sb, in_=t_bias.ap())
            idx_sb = cpool.tile([128, TOT // 16], dt.int16)
            nc.sync.dma_start(out=idx_sb, in_=t_idx.ap())
            adst_sb = cpool.tile([128, NW, HEADS], dt.bfloat16)
            nc.vector.memset(adst_sb, 0)

            # W_ext = [W | W @ att_flatT]  (264 cols, single phase-1 rhs)
            W_ext = cpool.tile([C_IN, C_OUT_TOT + 2 * HEADS], dt.bfloat16)
            nc.sync.dma_start(out=W_ext[:, 0:C_OUT_TOT], in_=t_Wb.ap())
            with tc.tile_pool(name="watt_ps", bufs=1, space="PSUM") as wpp, \
                 tc.tile_pool(name="watt_sb", bufs=1) as wsp:
                ps_watt = wpp.tile([C_IN, 2 * HEADS], dt.float32)
                wt0 = wsp.tile([128, C_IN], dt.bfloat16)
                wt1 = wsp.tile([128, C_IN], dt.bfloat16)
                at0 = wsp.tile([128, 2 * HEADS], dt.bfloat16)
                at1 = wsp.tile([128, 2 * HEADS], dt.bfloat16)
                nc.sync.dma_start(out=wt0, in_=t_WTb.ap()[0:128, :])
                nc.sync.dma_start(out=wt1, in_=t_WTb.ap()[128:256, :])
                nc.sync.dma_start(out=at0, in_=t_attT.ap()[0:128, :])
                nc.sync.dma_start(out=at1, in_=t_attT.ap()[128:256, :])
                nc.tensor.matmul(out=ps_watt, lhsT=wt0, rhs=at0, start=True, stop=False)
                nc.tensor.matmul(out=ps_watt, lhsT=wt1, rhs=at1, start=False, stop=True)
                nc.vector.tensor_copy(out=W_ext[:, C_OUT_TOT:C_OUT_TOT + 2 * HEADS],
                                      in_=ps_watt)

            # ---------- phase 1b FIRST: own-range h + a_src + a_dst ----------
            # own_h kept in SBUF: supplies a_dst logits and the self-loop
            # message at normalize time (self-loops are not gathered).
            own_h = cpool.tile([128, NW, C_OUT_TOT + HEADS], dt.bfloat16)
            nc.vector.memset(own_h, 0)
            p1bx = stack.enter_context(tc.tile_pool(name="p1bx", bufs=2))
            p1bps = stack.enter_context(tc.tile_pool(name="p1bps", bufs=1, space="PSUM"))
            if True:
                for w in range(NW):
                    nn = min(128, NPC - w * 128)
                    xo = p1bx.tile([128, 128], dt.bfloat16, tag="xo")
                    nc.sync.dma_start(out=xo[:, 0:nn],
                                      in_=t_xT_own.ap()[:, w * 128:w * 128 + nn])
                    ps_l2 = p1bps.tile([128, C_OUT_TOT + 2 * HEADS], dt.float32,
                                       tag="ps_l2")
                    nc.tensor.matmul(out=ps_l2[0:nn, :], lhsT=xo[:, 0:nn],
                                     rhs=W_ext, start=True, stop=True)
                    nc.scalar.copy(out=own_h[0:nn, w, :],
                                   in_=ps_l2[0:nn, 0:C_OUT_TOT + HEADS])
                    nc.vector.tensor_copy(
                        out=adst_sb[0:nn, w, :],
                        in_=ps_l2[0:nn, C_OUT_TOT + HEADS:C_OUT_TOT + 2 * HEADS])

            # ---------- phase 1: h table (single 264-col matmul per tile) ----
            # NOTE: phase-1 and phase-2 pools are opened in ONE scope below so
            # their SBUF zones are disjoint — zone reuse would make the first
            # gather wait for every phase-1 read (serializing the phases).
            lo_writes, hi_writes = [], []
            CHUNK = 6272  # 49 node-tiles per chunk
            p1x = stack.enter_context(tc.tile_pool(name="p1x", bufs=2))
            p1h = stack.enter_context(tc.tile_pool(name="p1h", bufs=6))
            p1ps = stack.enter_context(tc.tile_pool(name="p1ps", bufs=4, space="PSUM"))
            if True:
                ti_copy = 0
                for ci in range(0, N, CHUNK):
                    cw = min(CHUNK, N - ci)
                    xc = p1x.tile([128, CHUNK], dt.bfloat16, tag="xc")
                    nc.sync.dma_start(out=xc[:, 0:cw], in_=t_xT.ap()[:, ci:ci + cw])
                    for nt0 in range(0, cw, 128):
                        nn = min(128, cw - nt0)
                        ps_h = p1ps.tile([128, C_OUT_TOT + 2 * HEADS], dt.float32,
                                         tag="ps_h")
                        nc.tensor.matmul(out=ps_h[0:nn, :], lhsT=xc[:, nt0:nt0 + nn],
                                         rhs=W_ext, start=True, stop=True)
                        hsb = p1h.tile([128, ROW], dt.bfloat16, tag="hsb")
                        # 256 h + 4 a_src in one copy; cols 260:384 stay garbage
                        # (gathered but never read).
                        eng = nc.scalar if (ti_copy % 2 == 0) else nc.vector
                        if ti_copy % 2 == 0:
                            nc.scalar.copy(out=hsb[0:nn, 0:C_OUT_TOT + HEADS],
                                           in_=ps_h[0:nn, 0:C_OUT_TOT + HEADS])
                        else:
                            nc.vector.tensor_copy(
                                out=hsb[0:nn, 0:C_OUT_TOT + HEADS],
                                in_=ps_h[0:nn, 0:C_OUT_TOT + HEADS])
                        ti_copy += 1
                        n0 = ci + nt0
                        wi = nc.sync.dma_start(out=t_htab.ap()[n0:n0 + nn, :],
                                               in_=hsb[0:nn, :])
                        if n0 < SPLIT:
                            lo_writes.append(wi)
                        if n0 + nn > SPLIT:
                            hi_writes.append(wi)

            # ---------- phase 2 ----------
            # RAW fences: Tile does not track deps through DRAM tensors.
            # Lo gathers wait only on lo-half writes so they overlap the
            # hi-half table build.
            fence_lo = nc.sync.nop(hint="htab_fence_lo", nofuse=True)
            for _wi in lo_writes:
                _br.add_dep_helper(fence_lo.ins, _wi.ins, reason="htab lo RAW")
            fence_hi = nc.sync.nop(hint="htab_fence_hi", nofuse=True)
            for _wi in hi_writes:
                _br.add_dep_helper(fence_hi.ins, _wi.ins, reason="htab hi RAW")
            ap_lo = t_htab.ap()[0:SPLIT, :]
            ap_hi = t_htab.ap()[SPLIT:N, :]
            fences = (fence_lo, fence_hi)

            p2g = stack.enter_context(tc.tile_pool(name="p2g", bufs=4))
            p2o = stack.enter_context(tc.tile_pool(name="p2o", bufs=4))
            p2m = stack.enter_context(tc.tile_pool(name="p2m", bufs=2))
            p2s = stack.enter_context(tc.tile_pool(name="p2s", bufs=3))
            p2ps = stack.enter_context(tc.tile_pool(name="p2ps", bufs=2, space="PSUM"))
            p2pse = stack.enter_context(tc.tile_pool(name="p2pse", bufs=1, space="PSUM"))
            if True:
                for w in range(NW):
                    nn = min(128, NPC - w * 128)
                    T0, T1 = int(T_ws[w, 0]), int(T_ws[w, 1])
                    o0, o1 = int(off_ws[w, 0]), int(off_ws[w, 1])
                    Ts, os_ = (T0, T1), (o0, o1)
                    gb = [None, None]
                    for s in range(2):
                        T, o = Ts[s], os_[s]
                        gb[s] = p2g.tile([128, T_CAP, ROW], dt.bfloat16,
                                         tag=f"gb{s}", name=f"gb{s}")
                        _g = nc.gpsimd.dma_gather(
                            out_ap=gb[s][:, 0:T, :],
                            in_ap=(ap_lo if s == 0 else ap_hi),
                            idxs_ap=idx_sb[:, o // 16:(o + T * 128) // 16],
                            num_idxs=T * 128, num_idxs_reg=T * 128,
                            elem_size=ROW, single_packet=False,
                        )
                        _br.add_dep_helper(_g.ins, fences[s].ins, reason="htab RAW")
                    ohT_b = p2o.tile([128, T_CAP * 2 * 128], dt.float8e4, tag="ohT")
                    ohF_b = p2o.tile([128, T_CAP * 2 * 128], dt.float8e4, tag="ohF")
                    nc.sync.dma_start(out=ohT_b[:, 0:T0 * 128],
                                      in_=t_ohT.ap()[:, o0:o0 + T0 * 128])
                    nc.sync.dma_start(out=ohT_b[:, T0 * 128:(T0 + T1) * 128],
                                      in_=t_ohT.ap()[:, o1:o1 + T1 * 128])
                    nc.sync.dma_start(out=ohF_b[:, 0:T0 * 128],
                                      in_=t_ohF.ap()[:, o0:o0 + T0 * 128])
                    nc.sync.dma_start(out=ohF_b[:, T0 * 128:(T0 + T1) * 128],
                                      in_=t_ohF.ap()[:, o1:o1 + T1 * 128])

                    # a_dst per edge: [128e, 4] per tile
                    ps_adst = p2pse.tile([128, 2, T_CAP, HEADS], dt.float32,
                                         tag="ps_adst")
                    for s in range(2):
                        for t in range(Ts[s]):
                            nc.tensor.matmul(
                                out=ps_adst[:, s, t, :],
                                lhsT=ohF_b[:, (s * T0 + t) * 128:(s * T0 + t + 1) * 128],
                                rhs=adst_sb[:, w, :],
                                start=True, stop=True)

                    # e = exp(lrelu(a_src + a_dst)) -> msg[:, s, t, 256:260]
                    msg = p2m.tile([128, 2, T_CAP, 264], dt.bfloat16, tag="msg")
                    e_tmp = p2s.tile([128, 2, T_CAP, HEADS], dt.float32, tag="e_tmp")
                    e2 = p2s.tile([128, 2, T_CAP, HEADS], dt.float32, tag="e2")
                    for s in range(2):
                        T = Ts[s]
                        nc.vector.tensor_tensor(
                            out=e_tmp[:, s, 0:T, :], in0=ps_adst[:, s, 0:T, :],
                            in1=gb[s][:, 0:T, C_OUT_TOT:C_OUT_TOT + HEADS],
                            op=mybir.AluOpType.add)
                        nc.scalar.activation(out=e2[:, s, 0:T, :],
                                             in_=e_tmp[:, s, 0:T, :],
                                             func=mybir.ActivationFunctionType.Prelu,
                                             alpha=NEG_SLOPE)
                        nc.scalar.activation(out=msg[:, s, 0:T, 256:260],
                                             in_=e2[:, s, 0:T, :],
                                             func=mybir.ActivationFunctionType.Exp)

                    # msg = h * ex (broadcast per head)
                    for s in range(2):
                        for t in range(Ts[s]):
                            exb = msg[:, s, t, 256:260].unsqueeze(2).broadcast_to(
                                [128, HEADS, HC])
                            nc.vector.tensor_tensor(
                                out=msg[:, s, t, 0:C_OUT_TOT].rearrange(
                                    "p (h c) -> p h c", h=HEADS),
                                in0=gb[s][:, t, 0:C_OUT_TOT].rearrange(
                                    "p (h c) -> p h c", h=HEADS),
                                in1=exb, op=mybir.AluOpType.mult)

                    # aggregate: psum[p, 0:256] += msg, psum[p, 256:260] += ex
                    ps_win = p2ps.tile([128, 260], dt.float32, tag="ps_win")
                    n_t = T0 + T1
                    ti = 0
                    for s in range(2):
                        for t in range(Ts[s]):
                            nc.tensor.matmul(
                                out=ps_win,
                                lhsT=ohT_b[:, (s * T0 + t) * 128:(s * T0 + t + 1) * 128],
                                rhs=msg[:, s, t, 0:260],
                                start=(ti == 0), stop=(ti == n_t - 1))
                            ti += 1

                    # self-loop contribution from SBUF-resident own_h
                    sl_e = p2s.tile([128, HEADS], dt.float32, tag="sl_e")
                    nc.vector.tensor_tensor(
                        out=sl_e, in0=own_h[:, w, C_OUT_TOT:C_OUT_TOT + HEADS],
                        in1=adst_sb[:, w, :], op=mybir.AluOpType.add)
                    sl_p = p2s.tile([128, HEADS], dt.float32, tag="sl_p")
                    nc.scalar.activation(out=sl_p, in_=sl_e,
                                         func=mybir.ActivationFunctionType.Prelu,
                                         alpha=NEG_SLOPE)
                    slf = p2s.tile([128, HEADS], dt.float32, tag="slf")
                    nc.scalar.activation(out=slf, in_=sl_p,
                                         func=mybir.ActivationFunctionType.Exp)
                    nc.vector.tensor_tensor(out=ps_win[:, 256:260],
                                            in0=ps_win[:, 256:260], in1=slf,
                                            op=mybir.AluOpType.add)
                    slm = p2s.tile([128, C_OUT_TOT], dt.float32, tag="slm")
                    for h in range(HEADS):
                        nc.vector.tensor_scalar(
                            out=slm[:, h * HC:(h + 1) * HC],
                            in0=own_h[:, w, h * HC:(h + 1) * HC],
                            scalar1=slf[:, h:h + 1], scalar2=None,
                            op0=mybir.AluOpType.mult)
                    nc.vector.tensor_tensor(out=ps_win[:, 0:C_OUT_TOT],
                                            in0=ps_win[:, 0:C_OUT_TOT], in1=slm,
                                            op=mybir.AluOpType.add)

                    # normalize + bias
                    rcp = p2s.tile([128, HEADS], dt.float32, tag="rcp")
                    nc.vector.reciprocal(out=rcp, in_=ps_win[:, 256:260])
                    osb = p2s.tile([128, C_OUT_TOT], dt.float32, tag="osb")
                    for h in range(HEADS):
                        nc.vector.tensor_scalar(
                            out=osb[:, h * HC:(h + 1) * HC],
                            in0=ps_win[:, h * HC:(h + 1) * HC],
                            scalar1=rcp[:, h:h + 1], scalar2=None,
                            op0=mybir.AluOpType.mult)
                    nc.vector.tensor_tensor(out=osb, in0=osb, in1=bias_sb,
                                            op=mybir.AluOpType.add)
                    nc.sync.dma_start(out=t_out.ap()[w * 128:w * 128 + nn, :],
                                      in_=osb[0:nn, :])

    nc.finalize()
    return nc


def register_ntff_hook():
    import types
    import antenv
    if getattr(antenv, 'axon_hooks', None) is not None:
        return
    mod = types.ModuleType('antenv.axon_hooks')
    _hook = [None]
    mod.set_axon_ntff_profile_hook = lambda h: _hook.__setitem__(0, h)
    mod.get_axon_ntff_profile_hook = lambda: _hook[0]
    sys.modules['antenv.axon_hooks'] = mod
    antenv.axon_hooks = mod
    try:
        from trn_agent_boot.trn_boot import _ntff_profile_via_ctypes
        mod.set_axon_ntff_profile_hook(
            _ntff_profile_via_ctypes('/opt/axon/libaxon_pjrt.so'))
    except Exception:
        pass


def run(x, edge_index, W, att_src, att_dst, bias, n_cores=8, trace=False):
    cfg, in_maps = host_prep(x, edge_index, W, att_src, att_dst, bias, n_cores)
    nc = build_program(cfg)
    if trace:
        register_ntff_hook()
    r = bass_utils.run_bass_kernel_spmd(nc, in_maps,
                                        core_ids=list(range(n_cores)),
                                        trace=trace)
    out = np.concatenate([r.results[c]["out"] for c in range(n_cores)], axis=0)
    return out, r


import os as _os


def kernel(x, edge_index, W, att_src, att_dst, bias):
    x = np.asarray(x, np.float32)
    edge_index = np.asarray(edge_index)
    W = np.asarray(W, np.float32)
    att_src = np.asarray(att_src, np.float32)
    att_dst = np.asarray(att_dst, np.float32)
    bias = np.asarray(bias, np.float32)
    trace = _os.environ.get("GAT_TRACE", "0") == "1"
    out, r = run(x, edge_index, W, att_src, att_dst, bias, n_cores=8, trace=trace)
    if trace and r.exec_time_ns is not None:
        print(f"HW exec time: {r.exec_time_ns} ns")
    return np.ascontiguousarray(out.astype(np.float32))
